# revision 32
# baseline (speedup 1.0000x reference)
"""Cross-attention + RoPE Bass/Tile kernel for TRN2 (v2).

Per-core computation (batch element b = core id), all layouts transposed
host-side so every matmul contracts over the partition dim:

  xT   [C, NQ]   = x[b].T
  ctxT [C, NP]   = ctx[b].T
  wqT  [C, C]    = wq[permq].T   (permq: head-contiguous, per head 32
                                  even d's then 32 odd d's)
  wkT  [C, C]    = wk[permk].T   (permk: E/O split — per 128-row tile
                                  4 heads x 32 rows, even tiles hold
                                  even d's, odd tiles odd d's)
  wvT/woT [C, C] natural
  cos4/sin4 [128, NP] = freqs_cis[:, :, 0/1].T tiled 4x along partitions

Structure (single pass, everything SBUF-resident):
  ctx streamed in once on the ACT DMA queue; q-proj runs meanwhile.
  v = ctx.T @ wv  (per-128 k-chunk, + ones column per head)
  kT = wk.T @ ctx in E/O layout, RoPE'd in place (aligned vector ops),
       then repacked head-contiguous via 32 SBUF->SBUF DMAs.
  Attention per (head-pair, 512-q): scores in ONE K=64 matmul per
  (head, ki); exp with bias -12 (f16-safe, shift-invariant); AV
  accumulated in PSUM over ki with ps_s double-buffered so scores(ki+1)
  overlaps exp(ki) on the ACT engine.
  outT stays in SBUF; out-proj reads it directly.
"""

import sys as _s
if "/opt/trn_rl_repo" not in _s.path:
    _s.path.insert(0, "/opt/trn_rl_repo")

import numpy as np

import concourse.bass as bass
import concourse.mybir as mybir
from concourse.bass import ts

F32 = mybir.dt.float32
F16 = mybir.dt.float16

B, NQ, NP, C, H = 8, 1024, 2048, 1024, 16
D = C // H  # 64
NCT = C // 128  # 8 c-tiles
SCALE = 1.0 / np.sqrt(D)


def host_prep(x, ctx, freqs_cis, wq, bq, wk, bk, wv, bv, wo, bo):
    """Numpy-side layout prep. Returns per-core list of input dicts."""
    # permk: E/O split (RoPE-friendly): tile t = (g=t//2, odd=t%2),
    # slot -> head 4g + slot//32, d = 2*(slot%32) + odd
    permk = np.zeros(C, dtype=np.int64)
    for t in range(8):
        g, odd = t // 2, t % 2
        for slot in range(128):
            a, i = slot // 32, slot % 32
            permk[t * 128 + slot] = (4 * g + a) * 64 + 2 * i + odd
    # permq: head-contiguous, matching the post-repack kT layout:
    # head h rows = [d=0,2,..,62, then d=1,3,..,63]
    permq = np.zeros(C, dtype=np.int64)
    for h in range(H):
        for j in range(D):
            d = 2 * j if j < 32 else 2 * (j - 32) + 1
            permq[h * D + j] = h * D + d

    f32 = np.float32
    f16 = np.float16
    cosT = np.ascontiguousarray(freqs_cis[:, :, 0].T).astype(f16)  # [32, NP]
    sinT = np.ascontiguousarray(freqs_cis[:, :, 1].T).astype(f16)
    # softmax rows sum to 1, so attn@(v+bv) = attn@v + bv; the +bv then
    # folds through the out-projection: bo_eff = bo + wo @ bv (exact).
    bo_eff = np.asarray(bo, f32) + np.asarray(wo, f32) @ np.asarray(bv, f32)

    shared = {
        "wqT": np.ascontiguousarray(np.asarray(wq, f32)[permq].T.astype(f16)),
        "wkT": np.ascontiguousarray(np.asarray(wk, f32)[permk].T.astype(f16)),
        "wvT": np.ascontiguousarray(np.asarray(wv, f32).T.astype(f16)),
        "woT": np.ascontiguousarray(np.asarray(wo, f32).T.astype(f16)),
        "bq": np.asarray(bq, f32)[permq].copy(),
        "bk": np.asarray(bk, f32)[permk].copy(),
        "bo": bo_eff,
        "cos4": np.ascontiguousarray(np.tile(cosT, (4, 1))),
        "sin4": np.ascontiguousarray(np.tile(sinT, (4, 1))),
    }
    per_core = []
    for b in range(x.shape[0]):
        per_core.append({
            "xT": np.ascontiguousarray(np.asarray(x[b], f32).T.astype(f16)),
            "ctxT": np.ascontiguousarray(np.asarray(ctx[b], f32).T.astype(f16)),
            **shared,
        })
    return per_core


def declare_io(nc):
    """DRAM tensors; returns dict of APs."""
    d = {}
    d["xT"] = nc.dram_tensor("xT", [C, NQ], F16, kind="ExternalInput").ap()
    d["ctxT"] = nc.dram_tensor("ctxT", [C, NP], F16, kind="ExternalInput").ap()
    for w in ("wqT", "wkT", "wvT", "woT"):
        d[w] = nc.dram_tensor(w, [C, C], F16, kind="ExternalInput").ap()
    d["bq"] = nc.dram_tensor("bq", [C], F32, kind="ExternalInput").ap()
    d["bk"] = nc.dram_tensor("bk", [C], F32, kind="ExternalInput").ap()
    d["bo"] = nc.dram_tensor("bo", [C], F32, kind="ExternalInput").ap()
    d["cos4"] = nc.dram_tensor("cos4", [128, NP], F16, kind="ExternalInput").ap()
    d["sin4"] = nc.dram_tensor("sin4", [128, NP], F16, kind="ExternalInput").ap()
    d["y"] = nc.dram_tensor("y", [NQ, C], F32, kind="ExternalOutput").ap()
    return d


def emit(ctx, tc, io):
    """Emit the kernel under an open TileContext. ctx is an ExitStack."""
    nc = tc.nc
    Ident = mybir.ActivationFunctionType.Identity

    consts = ctx.enter_context(tc.tile_pool(name="consts", bufs=1))
    persist = ctx.enter_context(tc.tile_pool(name="persist", bufs=1))

    # --- constants -------------------------------------------------------
    bq_sb = consts.tile([128, NCT], F32, tag="bq", name="bq")
    bk_sb = consts.tile([128, NCT], F32, tag="bk", name="bk")
    nc.sync.dma_start(out=bq_sb[:], in_=io["bq"].rearrange("(t p) -> p t", p=128))
    nc.sync.dma_start(out=bk_sb[:], in_=io["bk"].rearrange("(t p) -> p t", p=128))
    nbias = consts.tile([128, 1], F32, tag="nbias", name="nbias")
    nc.vector.memset(nbias[:], -12.0)

    # --- persistent activations -----------------------------------------
    qTP = [persist.tile([128, NQ], F16, tag=f"qT{t}", name=f"qT{t}") for t in range(NCT)]
    kTP = [persist.tile([128, NP], F16, tag=f"kP{t}", name=f"kP{t}") for t in range(NCT)]
    outT = [persist.tile([128, NQ], F16, tag=f"oT{t}", name=f"oT{t}") for t in range(NCT)]
    v_sb = [
        persist.tile([128, H * (D + 1)], F16, tag=f"v{kc}", name=f"v{kc}")
        for kc in range(NP // 128)
    ]

    with tc.tile_pool(name="ctxp", bufs=1) as ctx_pool:
        # ctx streamed on the ACT DMA queue so phase1's weight loads (SP
        # queue) aren't stuck behind 4MB.
        ctx_sb = [
            ctx_pool.tile([128, NP], F16, tag=f"ctx{t}", name=f"ctx{t}")
            for t in range(NCT)
        ]
        with tc.tile_pool(name="wv", bufs=1) as wv_pool:
            wv_sb = [
                wv_pool.tile([128, C], F16, tag=f"wv{t}", name=f"wv{t}")
                for t in range(NCT)
            ]
            _phase1_qproj(tc, nc, io, qTP, bq_sb, Ident)
            # GPSIMD software-DGE queue: keeps big prefetches off the SP
            # queue (weights) and off the ACT queue (its SEQ blocks for the
            # whole transfer, which would stall PSUM drains). Emitted after
            # phase1 so x streams first; ctx/wv only needed at ~55us.
            for t in range(NCT):
                nc.gpsimd.dma_start(out=ctx_sb[t][:], in_=io["ctxT"][ts(t, 128), :])
            for t in range(NCT):
                nc.gpsimd.dma_start(out=wv_sb[t][:], in_=io["wvT"][ts(t, 128), :])
            with tc.tile_pool(name="keo", bufs=1) as keo_pool:
                kT = [
                    keo_pool.tile([128, NP], F16, tag=f"kT{t}", name=f"kT{t}")
                    for t in range(NCT)
                ]
                # K before V (ot-outer so rope can start per head-group
                # early); rope + repack then hide under phaseV's matmuls.
                _phaseK(tc, nc, io, ctx_sb, kT, bk_sb, Ident)
                _rope(tc, nc, io, kT)
                _repack_k(nc, kT, kTP)
            _phaseV(tc, nc, ctx_sb, wv_sb, v_sb)
    with tc.tile_pool(name="wo", bufs=1) as wo_pool:
        wo_sb = [
            wo_pool.tile([128, C], F16, tag=f"wo{t}", name=f"wo{t}")
            for t in range(NCT)
        ]
        for t in range(NCT):
            nc.sync.dma_start(out=wo_sb[t][:], in_=io["woT"][ts(t, 128), :])
        _phase4_attention(tc, nc, io, qTP, kTP, v_sb, outT, nbias, wo_sb)


def _phase1_qproj(tc, nc, io, qTP, bq_sb, Ident):
    with (
        tc.tile_pool(name="wq", bufs=1) as wq_pool,
        tc.tile_pool(name="xp", bufs=1) as x_pool,
        tc.tile_pool(name="ps1", bufs=4, space="PSUM") as ps1,
    ):
        wq_sb = [wq_pool.tile([128, C], F16, tag=f"wq{t}", name=f"wq{t}") for t in range(NCT)]
        x_sb = [x_pool.tile([128, NQ], F16, tag=f"x{t}", name=f"x{t}") for t in range(NCT)]
        for t in range(NCT):
            nc.sync.dma_start(out=wq_sb[t][:], in_=io["wqT"][ts(t, 128), :])
            nc.gpsimd.dma_start(out=x_sb[t][:], in_=io["xT"][ts(t, 128), :])
        for ot in range(NCT):
            for sh in range(NQ // 512):
                ps = ps1.tile([128, 512], F32, tag="ps", name="ps")
                for ct in range(NCT):
                    nc.tensor.matmul(
                        ps[:],
                        wq_sb[ct][:, ts(ot, 128)],
                        x_sb[ct][:, ts(sh, 512)],
                        start=(ct == 0),
                        stop=(ct == NCT - 1),
                    )
                nc.scalar.activation(
                    qTP[ot][:, ts(sh, 512)], ps[:], Ident,
                    bias=bq_sb[:, ot : ot + 1], scale=1.0,
                )


def _phaseV(tc, nc, ctx_sb, wv_sb, v_sb):
    Copy = mybir.ActivationFunctionType.Copy
    with tc.tile_pool(name="ps3", bufs=4, space="PSUM") as ps3:
        for kc in range(NP // 128):
            vv = v_sb[kc].rearrange("p (h c) -> p h c", c=D + 1)
            for oh in range(2):
                ps = ps3.tile([128, 512], F32, tag="ps", name="ps")
                for ct in range(NCT):
                    nc.tensor.matmul(
                        ps[:],
                        ctx_sb[ct][:, ts(kc, 128)],
                        wv_sb[ct][:, ts(oh, 512)],
                        start=(ct == 0),
                        stop=(ct == NCT - 1),
                    )
                # PSUM drain on ACT (DVE is busy with rope); bv is folded
                # into bo host-side (softmax rows sum to 1).
                nc.scalar.activation(
                    vv[:, oh * 8 : (oh + 1) * 8, 0:D],
                    ps.rearrange("p (h d) -> p h d", d=D),
                    Copy,
                )
            nc.vector.memset(vv[:, :, D : D + 1], 1.0)


def _phaseK(tc, nc, io, ctx_sb, kT, bk_sb, Ident):
    with (
        tc.tile_pool(name="wk", bufs=1) as wk_pool,
        tc.tile_pool(name="ps2", bufs=4, space="PSUM") as ps2,
    ):
        wk_sb = [wk_pool.tile([128, C], F16, tag=f"wk{t}", name=f"wk{t}") for t in range(NCT)]
        for t in range(NCT):
            nc.sync.dma_start(out=wk_sb[t][:], in_=io["wkT"][ts(t, 128), :])
        for ot in range(NCT):
            for ks in range(NP // 512):
                ps = ps2.tile([128, 512], F32, tag="ps", name="ps")
                for ct in range(NCT):
                    nc.tensor.matmul(
                        ps[:],
                        wk_sb[ct][:, ts(ot, 128)],
                        ctx_sb[ct][:, ts(ks, 512)],
                        start=(ct == 0),
                        stop=(ct == NCT - 1),
                    )
                nc.scalar.activation(
                    kT[ot][:, ts(ks, 512)], ps[:], Ident,
                    bias=bk_sb[:, ot : ot + 1], scale=1.0,
                )


def _rope(tc, nc, io, kT):
    with tc.tile_pool(name="rope", bufs=1) as rope_pool:
        cos4 = rope_pool.tile([128, NP], F16, tag="cos4", name="cos4")
        sin4 = rope_pool.tile([128, NP], F16, tag="sin4", name="sin4")
        nc.sync.dma_start(out=cos4[:], in_=io["cos4"][:])
        nc.sync.dma_start(out=sin4[:], in_=io["sin4"][:])
        for g in range(4):
            E, O = kT[2 * g], kT[2 * g + 1]
            tA = rope_pool.tile([128, NP], F16, tag="tA", name="tA")
            tB = rope_pool.tile([128, NP], F16, tag="tB", name="tB")
            nc.vector.tensor_mul(tA[:], O[:], sin4[:])
            nc.vector.tensor_mul(tB[:], O[:], cos4[:])
            nc.vector.tensor_mul(O[:], E[:], sin4[:])
            nc.vector.tensor_mul(E[:], E[:], cos4[:])
            nc.vector.tensor_sub(E[:], E[:], tA[:])
            nc.vector.tensor_add(O[:], O[:], tB[:])


def _repack_k(nc, kT, kTP):
    # head h: E rows kT[2*(h//4)][32*(h%4):+32], O rows same in the odd
    # tile -> kTP[h//2][(h%2)*64 : +64] = [E; O]
    for h in range(H):
        g, a = h // 4, h % 4
        p, off = h // 2, (h % 2) * 64
        nc.sync.dma_start(
            out=kTP[p][off : off + 32, :], in_=kT[2 * g][32 * a : 32 * a + 32, :]
        )
        nc.sync.dma_start(
            out=kTP[p][off + 32 : off + 64, :],
            in_=kT[2 * g + 1][32 * a : 32 * a + 32, :],
        )


def _phase4_attention(tc, nc, io, qTP, kTP, v_sb, outT, nbias, wo_sb):
    Exp = mybir.ActivationFunctionType.Exp
    with (
        tc.tile_pool(name="expp", bufs=3) as exp_pool,
        tc.tile_pool(name="stg", bufs=2) as stg_pool,
        tc.tile_pool(name="yacc", bufs=1) as y_pool,
        tc.tile_pool(name="ps_s", bufs=2, space="PSUM") as ps_s_pool,
        tc.tile_pool(name="ps_o", bufs=1, space="PSUM") as ps_o_pool,
        tc.tile_pool(name="ps_y", bufs=2, space="PSUM") as ps_y_pool,
    ):
        # out-proj is folded into phase4: as each head-pair's outT columns
        # land, its 16 y-chunk matmuls run in phase4's PE slack (ACT-bound)
        # and accumulate into y_acc on the mostly-idle DVE.
        bo_row = y_pool.tile([1, C], F32, tag="bo_row", name="bo_row")
        nc.sync.dma_start(out=bo_row[:], in_=io["bo"].unsqueeze(0))
        bo_rep = y_pool.tile([128, C], F32, tag="bo_rep", name="bo_rep")
        nc.gpsimd.partition_broadcast(bo_rep[:], bo_row[:], channels=128)
        y_acc = [
            y_pool.tile([128, C], F32, tag=f"ya{sc}", name=f"ya{sc}")
            for sc in range(NQ // 128)
        ]
        for t in range(NCT):
            for qh in range(NQ // 512):
                ps_o = [
                    ps_o_pool.tile([D + 1, 512], F32, tag=f"ps_o{hh}", name=f"ps_o{hh}")
                    for hh in range(2)
                ]
                NKI = NP // 128

                def scores(ki):
                    ps_s = ps_s_pool.tile([128, 1024], F32, tag="ps_s", name="ps_s")
                    for hh in range(2):
                        nc.tensor.matmul(
                            ps_s[:, ts(hh, 512)],
                            kTP[t][64 * hh : 64 * hh + 64, ts(ki, 128)],
                            qTP[t][64 * hh : 64 * hh + 64, ts(qh, 512)],
                            start=True,
                            stop=True,
                        )
                    return ps_s

                # software-pipelined: s(ki+1) is emitted before av(ki) so the
                # in-order PE queue runs it during exp(ki) instead of
                # stalling behind av(ki)'s wait on the exp result.
                ps_s = scores(0)
                for ki in range(NKI):
                    expT = exp_pool.tile([128, 1024], F16, tag="expT", name="expT")
                    # bias=-12 keeps exp() in f16 range (max scaled score
                    # ~16.4); softmax is shift-invariant so it divides out.
                    nc.scalar.activation(
                        expT[:], ps_s[:], Exp, bias=nbias[:], scale=float(SCALE)
                    )
                    if ki + 1 < NKI:
                        ps_s = scores(ki + 1)
                    vv = v_sb[ki].rearrange("p (h c) -> p h c", c=D + 1)
                    for hh in range(2):
                        nc.tensor.matmul(
                            ps_o[hh][:],
                            vv[:, 2 * t + hh, :],
                            expT[:, ts(hh, 512)],
                            start=(ki == 0),
                            stop=(ki == NKI - 1),
                        )
                for hh in range(2):
                    # copy ps_o out first so its PSUM bank frees for the next
                    # group's AV before the recip->broadcast->mul chain runs
                    onrm = stg_pool.tile([D + 1, 512], F32, tag="onrm", name="onrm")
                    nc.vector.tensor_copy(onrm[:], ps_o[hh][:])
                    recip = stg_pool.tile([1, 512], F32, tag="recip", name="recip")
                    nc.vector.reciprocal(recip[:], onrm[D : D + 1, :])
                    bc = stg_pool.tile([D, 512], F32, tag="bc", name="bc")
                    nc.gpsimd.partition_broadcast(bc[:], recip[:], channels=D)
                    stage = stg_pool.tile([D, 512], F16, tag="stage", name="stage")
                    nc.vector.tensor_mul(stage[:], onrm[0:D, :], bc[:])
                    off = 64 * hh
                    nc.sync.dma_start(
                        out=outT[t][off : off + 64, ts(qh, 512)], in_=stage[:]
                    )

            # out-proj contribution of head-pair t, accumulated into y_acc;
            # runs in phase4's PE slack, overlapping later groups.
            for sc in range(NQ // 128):
                for oh in range(2):
                    ps = ps_y_pool.tile([128, 512], F32, tag="ps_y", name="ps_y")
                    nc.tensor.matmul(
                        ps[:],
                        outT[t][:, ts(sc, 128)],
                        wo_sb[t][:, ts(oh, 512)],
                        start=True,
                        stop=True,
                    )
                    ya = y_acc[sc][:, ts(oh, 512)]
                    if t == 0:
                        nc.vector.tensor_add(ya, ps[:], bo_rep[:, ts(oh, 512)])
                    else:
                        nc.vector.tensor_add(ya, ya, ps[:])
                if t == NCT - 1:
                    nc.sync.dma_start(out=io["y"][ts(sc, 128), :], in_=y_acc[sc][:])


# ======================================================================
# Self-contained entry point: kernel(**inputs) with FULL unsharded inputs.
# Shards batch across 8 NeuronCores (data parallel), runs the Bass kernel
# via run_bass_kernel_spmd, returns the FULL [8, 1024, 1024] output.
# ======================================================================

import sys as _sys
if "/opt/trn_rl_repo" not in _sys.path:
    _sys.path.insert(0, "/opt/trn_rl_repo")

_NC_CACHE = {}


def _build_nc():
    if "nc" in _NC_CACHE:
        return _NC_CACHE["nc"]
    from contextlib import ExitStack
    import concourse.tile as tile
    from concourse import bacc

    nc = bacc.Bacc("TRN2", target_bir_lowering=False, debug=False, num_devices=B)
    io = declare_io(nc)
    with tile.TileContext(nc, trace_sim=False) as tc:
        with ExitStack() as st:
            emit(st, tc, io)
    nc.compile()
    _NC_CACHE["nc"] = nc
    return nc


def kernel(x, ctx, freqs_cis, wq, bq, wk, bk, wv, bv, wo, bo):
    from concourse import bass_utils

    nc = _build_nc()
    in_maps = host_prep(x, ctx, freqs_cis, wq, bq, wk, bk, wv, bv, wo, bo)
    res = bass_utils.run_bass_kernel_spmd(
        nc, in_maps, core_ids=list(range(len(in_maps))), trace=False
    )
    return np.stack([res.results[b]["y"] for b in range(len(in_maps))]).astype(
        np.float32
    )
